# revision 1
# baseline (speedup 1.0000x reference)
"""2-layer GAT (nn_GATNet) on 8 TRN2 NeuronCores — self-contained kernel.

Architecture (SPMD, one program on 8 cores, dst-node sharding 6250/core):
  phase A1 (replicated): h_ext1[n] = [x@W1 | x@Wa1s | x@Wa1d] for all nodes,
      written to a DRAM table ([N, 80] fp32). Alpha terms are folded into the
      weight matrix on host (W_ext = [W | W.a_src | W.a_dst]).
  phase B1 (edge phase): edges (+self loops) sorted by dst, sharded by dst
      range; per 128-dst window, T=18 tiles of 128 edge slots (uniform
      schedule across cores; padded slots use src=0 with a one-hot offset that
      matches nothing). Per tile: indirect-DMA gather of h_ext1[src] rows;
      one-hot(dstoff) via is_equal against an iota matrix; alpha_dst expanded
      from a dense window slice via PE transpose(one-hot) + matmul; logits ->
      leaky_relu -> exp; segment softmax realized as U = sum(ex*h), denom =
      sum(ex) accumulated in PSUM by one-hot matmuls, then agg = U/denom.
      Softmax max-subtraction is omitted (ratio-invariant; logits bounded).
  phase A2: h2 = elu(h1)@W_ext2 for the local shard, AllGather -> h2 table.
  phase B2: same edge phase with 1 head / 40 dims, then log_softmax, output
      shard [6250, 40]; host concatenates shards.
"""
import numpy as np
import concourse.bass as bass
import concourse.bacc as bacc
import concourse.tile as tile
from concourse import mybir
from concourse.bass_utils import run_bass_kernel_spmd

P = 128
F32 = mybir.dt.float32
I32 = mybir.dt.int32
AF = mybir.ActivationFunctionType
OP = mybir.AluOpType
PADOFF = 200.0

N_NODES = 50000
NC = 8
ST = 6


def _fold_params(W1, a1_src, a1_dst, W2, a2_src, a2_dst):
    def fold(W, a):
        heads, od = a.shape
        return np.einsum("cho,ho->ch", W.reshape(W.shape[0], heads, od), a)
    W_ext1 = np.concatenate([W1, fold(W1, a1_src), fold(W1, a1_dst)], axis=1)
    W_ext2 = np.concatenate([W2, fold(W2, a2_src), fold(W2, a2_dst)], axis=1)
    return (np.ascontiguousarray(W_ext1, np.float32),
            np.ascontiguousarray(W_ext2, np.float32))


def _prep_edges(src, dst, N, T):
    shard = N // NC
    NW = (shard + P - 1) // P
    NT = NW * T
    per_core = []
    for c in range(NC):
        lo = c * shard
        m = (dst >= lo) & (dst < lo + shard)
        s_c = src[m].astype(np.int32)
        ld = (dst[m] - lo).astype(np.int32)
        order = np.argsort(ld, kind="stable")
        s_c, ld = s_c[order], ld[order]
        win = ld >> 7
        off = (ld & 127).astype(np.float32)
        src_idx = np.zeros((NT, P), np.int32)
        dstoff = np.full((NT, P), PADOFF, np.float32)
        wstart = np.searchsorted(win, np.arange(NW + 1))
        for w in range(NW):
            a, b = wstart[w], wstart[w + 1]
            cnt = b - a
            assert cnt <= T * P, f"window overflow: {cnt} > {T * P}"
            src_idx[w * T:(w + 1) * T].reshape(-1)[:cnt] = s_c[a:b]
            dstoff[w * T:(w + 1) * T].reshape(-1)[:cnt] = off[a:b]
        dwin = np.zeros((NW, P), np.int32)
        for w in range(NW):
            ids = lo + w * P + np.arange(P)
            ids[ids >= lo + shard] = 0
            dwin[w] = ids
        per_core.append((np.ascontiguousarray(src_idx.T),
                         np.ascontiguousarray(dstoff.T),
                         np.ascontiguousarray(dwin.T)))
    return per_core, NW, NT


def build_kernel(N, T, reps=1):
    shard = N // NC
    NW = (shard + P - 1) // P
    NT = NW * T
    NTA = (N + P - 1) // P
    W1O, W2O = 80, 42
    CH = 16
    NSUP = (T + ST - 1) // ST

    nc = bacc.Bacc("TRN2", target_bir_lowering=False, debug=False)

    xT = nc.dram_tensor("xT", [P, N], F32, kind="ExternalInput")
    W_ext1 = nc.dram_tensor("W_ext1", [P, W1O], F32, kind="ExternalInput")
    W_ext2 = nc.dram_tensor("W_ext2", [64, W2O], F32, kind="ExternalInput")
    b1m = nc.dram_tensor("b1m", [P, 64], F32, kind="ExternalInput")
    b2m = nc.dram_tensor("b2m", [P, 40], F32, kind="ExternalInput")
    iota_in = nc.dram_tensor("iota_in", [P, P], F32, kind="ExternalInput")
    ident_in = nc.dram_tensor("ident_in", [P, P], F32, kind="ExternalInput")
    src_idx = nc.dram_tensor("src_idx", [P, NT], I32, kind="ExternalInput")
    dstoff_in = nc.dram_tensor("dstoff_in", [P, NT], F32, kind="ExternalInput")
    dwin_in = nc.dram_tensor("dwin_in", [P, NW], I32, kind="ExternalInput")
    out = nc.dram_tensor("out", [shard, 40], F32, kind="ExternalOutput")

    hext1 = nc.dram_tensor("hext1", [N, W1O], F32)
    h2_shard = nc.dram_tensor("h2_shard", [shard, W2O], F32)
    hext2 = nc.dram_tensor("hext2", [N, W2O], F32, addr_space="Shared")

    with tile.TileContext(nc) as tc:
        cp = tc.alloc_tile_pool(name="const", bufs=1)
        w1_sb = cp.tile([P, W1O], F32)
        nc.sync.dma_start(out=w1_sb[:], in_=W_ext1[:])
        w2_sb = cp.tile([64, W2O], F32)
        nc.sync.dma_start(out=w2_sb[:], in_=W_ext2[:])
        b1_sb = cp.tile([P, 64], F32)
        nc.sync.dma_start(out=b1_sb[:], in_=b1m[:])
        b2_sb = cp.tile([P, 40], F32)
        nc.sync.dma_start(out=b2_sb[:], in_=b2m[:])
        iota_sb = cp.tile([P, P], F32)
        nc.sync.dma_start(out=iota_sb[:], in_=iota_in[:])
        ident_sb = cp.tile([P, P], F32)
        nc.sync.dma_start(out=ident_sb[:], in_=ident_in[:])
        sidx_sb = cp.tile([P, NT], I32)
        nc.sync.dma_start(out=sidx_sb[:], in_=src_idx[:])
        doff_sb = cp.tile([P, NT], F32)
        nc.sync.dma_start(out=doff_sb[:], in_=dstoff_in[:])
        dwin_sb = cp.tile([P, NW], I32)
        nc.sync.dma_start(out=dwin_sb[:], in_=dwin_in[:])
        h1act_sb = cp.tile([P, NW * 64], F32)
        uall_sb = cp.tile([P, NW, 80], F32)     # U copies, max(HC+NH)
        hself_sb = cp.tile([P, NW, 64], F32)    # window-node h rows
        es_sb = cp.tile([P, NW, 8], F32)        # self-loop logits

        gp = tc.alloc_tile_pool(name="gp", bufs=6)
        ohp = tc.alloc_tile_pool(name="ohp", bufs=3)
        rp = tc.alloc_tile_pool(name="rp", bufs=3)
        sp = tc.alloc_tile_pool(name="sp", bufs=6)
        fp = tc.alloc_tile_pool(name="fin", bufs=1)
        pu = tc.alloc_tile_pool(name="pu", bufs=2, space="PSUM")
        pt = tc.alloc_tile_pool(name="pt", bufs=2, space="PSUM")
        pe = tc.alloc_tile_pool(name="pe", bufs=2, space="PSUM")

        def gather(dest_ap, table, idx_col):
            nc.gpsimd.indirect_dma_start(
                out=dest_ap, out_offset=None, in_=table[:],
                in_offset=bass.IndirectOffsetOnAxis(ap=idx_col, axis=0))

        def edge_phase(table, WROW, NH, OD, post):
            HC = NH * OD
            for w in range(NW):
                ad_g = sp.tile([P, WROW], F32, tag="ad_g")
                gather(ad_g[:], table, dwin_sb[:, w:w + 1])
                nc.vector.tensor_add(out=es_sb[:, w, 0:NH],
                                     in0=ad_g[:, HC:HC + NH],
                                     in1=ad_g[:, HC + NH:HC + 2 * NH])
                nc.scalar.activation(out=hself_sb[:, w, 0:HC], in_=ad_g[:, 0:HC],
                                     func=AF.Copy)
                U_ps = pu.tile([P, HC + NH], F32, space="PSUM", tag="U")
                for st in range(NSUP):
                    t0 = w * T + st * ST
                    STc = min(ST, T - st * ST)
                    g_b = gp.tile([P, STc, WROW], F32, tag="g_b")
                    for tt in range(STc):
                        gather(g_b[:, tt, :], table, sidx_sb[:, t0 + tt:t0 + tt + 1])
                    oh_b = ohp.tile([P, STc, P], F32, tag="oh_b")
                    nc.vector.tensor_tensor(
                        out=oh_b[:],
                        in0=doff_sb[:, t0:t0 + STc, None].to_broadcast([P, STc, P]),
                        in1=iota_sb[:, None, :].to_broadcast([P, STc, P]),
                        op=OP.is_equal)
                    ade_ps = pe.tile([P, STc * NH], F32, space="PSUM", tag="ade")
                    for tt in range(STc):
                        ohT_ps = pt.tile([P, P], F32, space="PSUM", tag="ohT")
                        nc.tensor.transpose(out=ohT_ps[:], in_=oh_b[:, tt, :],
                                            identity=ident_sb[:])
                        ohT_sb = sp.tile([P, P], F32, tag="ohT_sb")
                        nc.scalar.activation(out=ohT_sb[:], in_=ohT_ps[:], func=AF.Copy)
                        nc.tensor.matmul(
                            out=ade_ps[:, tt * NH:(tt + 1) * NH], lhsT=ohT_sb[:],
                            rhs=ad_g[:, HC + NH:HC + 2 * NH], start=True, stop=True)
                    e_b = sp.tile([P, STc, NH], F32, tag="e_b")
                    nc.vector.tensor_add(
                        out=e_b[:], in0=g_b[:, :, HC:HC + NH],
                        in1=ade_ps[:].rearrange("p (s h) -> p s h", h=NH))
                    l_b = sp.tile([P, STc, NH], F32, tag="l_b")
                    nc.vector.scalar_tensor_tensor(
                        out=l_b[:], in0=e_b[:], scalar=0.2, in1=e_b[:],
                        op0=OP.mult, op1=OP.max)
                    rhs_b = rp.tile([P, STc, HC + NH], F32, tag="rhs_b")
                    nc.scalar.activation(out=rhs_b[:, :, HC:HC + NH], in_=l_b[:],
                                         func=AF.Exp)
                    nc.vector.tensor_tensor(
                        out=rhs_b[:, :, 0:HC].rearrange("p s (h o) -> p s h o", o=OD),
                        in0=g_b[:, :, 0:HC].rearrange("p s (h o) -> p s h o", o=OD),
                        in1=rhs_b[:, :, HC:HC + NH, None].to_broadcast([P, STc, NH, OD]),
                        op=OP.mult)
                    for tt in range(STc):
                        nc.tensor.matmul(
                            out=U_ps[:], lhsT=oh_b[:, tt, :], rhs=rhs_b[:, tt, :],
                            start=(st == 0 and tt == 0),
                            stop=(st == NSUP - 1 and tt == STc - 1))
                nc.scalar.activation(out=uall_sb[:, w, 0:HC + NH], in_=U_ps[:],
                                     func=AF.Copy)
            post()

        def finish(NH, OD):
            """Batched over all windows: self terms + normalize -> agg."""
            HC = NH * OD
            ls = fp.tile([P, NW, NH], F32, tag="F")
            nc.vector.scalar_tensor_tensor(
                out=ls[:], in0=es_sb[:, :, 0:NH], scalar=0.2,
                in1=es_sb[:, :, 0:NH], op0=OP.mult, op1=OP.max)
            exs = fp.tile([P, NW, NH], F32, tag="G")
            nc.scalar.activation(out=exs[:], in_=ls[:], func=AF.Exp)
            den = fp.tile([P, NW, NH], F32, tag="H")
            nc.vector.tensor_add(out=den[:], in0=uall_sb[:, :, HC:HC + NH],
                                 in1=exs[:])
            Uf = fp.tile([P, NW, HC], F32, tag="A")
            nc.vector.tensor_tensor(
                out=Uf[:].rearrange("p w (h o) -> p w h o", o=OD),
                in0=hself_sb[:, :, 0:HC].rearrange("p w (h o) -> p w h o", o=OD),
                in1=exs[:, :, :, None].to_broadcast([P, NW, NH, OD]), op=OP.mult)
            nc.vector.tensor_add(out=Uf[:], in0=Uf[:],
                                 in1=uall_sb[:, :, 0:HC])
            recip = fp.tile([P, NW, NH], F32, tag="I")
            nc.vector.reciprocal(recip[:], den[:])
            agg = fp.tile([P, NW, HC], F32, tag="B")
            nc.vector.tensor_tensor(
                out=agg[:].rearrange("p w (h o) -> p w h o", o=OD),
                in0=Uf[:].rearrange("p w (h o) -> p w h o", o=OD),
                in1=recip[:, :, :, None].to_broadcast([P, NW, NH, OD]),
                op=OP.mult)
            return agg

        def post1():
            agg = finish(8, 8)
            nc.vector.tensor_add(
                out=agg[:], in0=agg[:],
                in1=b1_sb[:, None, :].to_broadcast([P, NW, 64]))
            ex1 = fp.tile([P, NW, 64], F32, tag="C")
            nc.scalar.activation(out=ex1[:], in_=agg[:], func=AF.Exp)
            em = fp.tile([P, NW, 64], F32, tag="D")
            nc.vector.tensor_scalar(out=em[:], in0=ex1[:], scalar1=-1.0,
                                    scalar2=0.0, op0=OP.add, op1=OP.min)
            nc.vector.scalar_tensor_tensor(
                out=h1act_sb[:].rearrange("p (w f) -> p w f", f=64),
                in0=agg[:], scalar=0.0, in1=em[:], op0=OP.max, op1=OP.add)

        def post2():
            agg = finish(1, 40)
            nc.vector.tensor_add(
                out=agg[:], in0=agg[:],
                in1=b2_sb[:, None, :].to_broadcast([P, NW, 40]))
            mx = fp.tile([P, NW, 1], F32, tag="J")
            nc.vector.reduce_max(out=mx[:], in_=agg[:], axis=mybir.AxisListType.X)
            tm = fp.tile([P, NW, 40], F32, tag="D")
            nc.vector.tensor_sub(out=tm[:], in0=agg[:],
                                 in1=mx[:, :, 0:1].to_broadcast([P, NW, 40]))
            q = fp.tile([P, NW, 40], F32, tag="C")
            nc.scalar.activation(out=q[:], in_=tm[:], func=AF.Exp)
            s = fp.tile([P, NW, 1], F32, tag="J")
            nc.vector.reduce_sum(out=s[:], in_=q[:], axis=mybir.AxisListType.X)
            lsf = fp.tile([P, NW, 1], F32, tag="I")
            nc.scalar.activation(out=lsf[:], in_=s[:], func=AF.Ln)
            o = fp.tile([P, NW, 40], F32, tag="E")
            nc.vector.tensor_sub(out=o[:], in0=tm[:],
                                 in1=lsf[:, :, 0:1].to_broadcast([P, NW, 40]))
            for w in range(NW):
                rows = min(P, shard - w * P)
                nc.sync.dma_start(out=out[w * P:w * P + rows, :],
                                  in_=o[:rows, w, :])

        for rep in range(reps):
            with (tc.tile_pool(name="xa", bufs=2) as xa,
                  tc.tile_pool(name="ha", bufs=3) as ha,
                  tc.tile_pool(name="pa", bufs=2, space="PSUM") as pa):
                for ch in range(0, NTA, CH):
                    ntile = min(CH, NTA - ch)
                    cols = min(CH * P, N - ch * P)
                    xc = xa.tile([P, CH * P], F32, tag="xc")
                    nc.sync.dma_start(out=xc[:, :cols], in_=xT[:, ch * P:ch * P + cols])
                    for t in range(ntile):
                        n0 = (ch + t) * P
                        rows = min(P, N - n0)
                        ps = pa.tile([P, W1O], F32, space="PSUM", tag="psA")
                        nc.tensor.matmul(out=ps[:rows, :],
                                         lhsT=xc[:, t * P:t * P + rows],
                                         rhs=w1_sb[:], start=True, stop=True)
                        hb = ha.tile([P, W1O], F32, tag="hb")
                        nc.scalar.activation(out=hb[:rows, :], in_=ps[:rows, :],
                                             func=AF.Copy)
                        nc.sync.dma_start(out=hext1[n0:n0 + rows, :], in_=hb[:rows, :])

            edge_phase(hext1, W1O, 8, 8, post1)

            with (tc.tile_pool(name="a2", bufs=3) as a2,
                  tc.tile_pool(name="p2", bufs=1, space="PSUM") as p2):
                for w in range(NW):
                    rows = min(P, shard - w * P)
                    hT_ps = p2.tile([64, P], F32, space="PSUM", tag="hT")
                    nc.tensor.transpose(out=hT_ps[:],
                                        in_=h1act_sb[:, w * 64:(w + 1) * 64],
                                        identity=ident_sb[:])
                    hT_sb = a2.tile([64, P], F32, tag="hT_sb")
                    nc.scalar.activation(out=hT_sb[:], in_=hT_ps[:], func=AF.Copy)
                    ps2 = p2.tile([P, W2O], F32, space="PSUM", tag="ps2")
                    nc.tensor.matmul(out=ps2[:], lhsT=hT_sb[:], rhs=w2_sb[:],
                                     start=True, stop=True)
                    h2b = a2.tile([P, W2O], F32, tag="h2b")
                    nc.scalar.activation(out=h2b[:], in_=ps2[:], func=AF.Copy)
                    nc.sync.dma_start(out=h2_shard[w * P:w * P + rows, :],
                                      in_=h2b[:rows, :])
            nc.gpsimd.collective_compute(
                "AllGather", OP.bypass, replica_groups=[list(range(NC))],
                ins=[h2_shard[:]], outs=[hext2[:]])

            edge_phase(hext2, W2O, 1, 40, post2)

        for pool in (pe, pt, pu, fp, sp, rp, ohp, gp, cp):
            pool.release()

    nc.compile()
    return nc


_CACHE = {}


def _get_nc(T, reps=1):
    key = (T, reps)
    if key not in _CACHE:
        _CACHE[key] = build_kernel(N_NODES, T, reps=reps)
    return _CACHE[key]


def make_in_maps(x, edge_index, W1, a1_src, a1_dst, b1, W2, a2_src, a2_dst, b2, T,
                 N=None):
    N = N or N_NODES
    W_ext1, W_ext2 = _fold_params(W1, a1_src, a1_dst, W2, a2_src, a2_dst)
    src = np.asarray(edge_index[0]).astype(np.int64)
    dst = np.asarray(edge_index[1]).astype(np.int64)
    per_core, NW, NT = _prep_edges(src, dst, N, T)
    shared = {
        "xT": np.ascontiguousarray(x.T, np.float32),
        "W_ext1": W_ext1, "W_ext2": W_ext2,
        "b1m": np.tile(np.asarray(b1, np.float32)[None, :], (P, 1)),
        "b2m": np.tile(np.asarray(b2, np.float32)[None, :], (P, 1)),
        "iota_in": np.tile(np.arange(P, dtype=np.float32), (P, 1)),
        "ident_in": np.eye(P, dtype=np.float32),
    }
    return [dict(shared, src_idx=si, dstoff_in=do, dwin_in=dw)
            for (si, do, dw) in per_core]


def required_T(edge_index, N=None):
    N = N or N_NODES
    dst = np.asarray(edge_index[1]).astype(np.int64)
    shard = N // NC
    maxt = 1
    for c in range(NC):
        ld = dst[(dst >= c * shard) & (dst < (c + 1) * shard)] - c * shard
        wc = np.bincount(ld >> 7, minlength=(shard + P - 1) // P)
        maxt = max(maxt, int(np.ceil(wc.max() / P)))

    return ((maxt + ST - 1) // ST) * ST


def kernel(x, edge_index, W1, a1_src, a1_dst, b1, W2, a2_src, a2_dst, b2,
           reps=1, nc_override=None):
    x = np.asarray(x, np.float32)
    edge_index = np.asarray(edge_index)
    args = [np.asarray(a, np.float32) for a in
            (W1, a1_src, a1_dst, b1, W2, a2_src, a2_dst, b2)]
    T = required_T(edge_index)
    in_maps = make_in_maps(x, edge_index, *args, T)
    nc = nc_override if nc_override is not None else _get_nc(T, reps)
    res = run_bass_kernel_spmd(nc, in_maps, list(range(NC)))
    return np.concatenate([res.results[c]["out"] for c in range(NC)], axis=0)



# revision 8
# speedup vs baseline: 21.8260x; 21.8260x over previous
"""2-layer GAT (nn_GATNet) on 8 TRN2 NeuronCores — self-contained kernel.

Design (SPMD, one program on 8 cores, dst-node sharding 6250/core).

The runtime here charges ~40-75us per STATIC instruction, so the kernel is
built around hardware For_i loops with tiny bodies (~150 static instructions
total) and a matmul-free edge phase:

  A1 (For_i over 49 windows): t1_shard = x @ [W1 | W1 a1_src | W1 a1_dst]
     for the local dst shard ([6250,128] rows, 80 cols used, 512B rows),
     then AllGather -> t1 [50000,128] on every core.
  B1 (For_i over windows): K-binned dst-major edge layout. Each window = 128
     dst nodes (one per partition); bin k of dst d sits at gather slot
     k*128+d, so dma_gather (non-transpose) lands each dst's K bins on its
     own partition: g[d, k, :]. Bins are split into a lo section (src <
     32768, K1 bins) and a hi section (src-32768, K2 bins) because dma_gather
     indices are int16; hi gathers use a base-shifted table AP. Self loops
     are ordinary bins; pad bins gather row 0 and carry a -300 logit mask.
     Per window: ~7 dma_gathers (<=1024 idxs each, the ucode ring cap) +
     ~14 DVE/Act ops: logits = g[:,:,acol:acol+NH] + alpha_dst (free-dim
     broadcast from a direct window DMA of the local shard rows) + mask;
     ex = exp(leakyrelu); msg = g[:,:,0:HC]*ex; U/denom = free-axis reduce
     over k; agg = U/denom. No PE, no one-hot, no transposes.
  A2 (For_i): t2_shard = elu(h1) @ [W2 | ...] ([6250,64] rows, 42 used,
     256B rows), AllGather -> t2.
  B2: same edge phase with 1 head / 40 dims + log_softmax, writes the local
     [6250, 40] output shard; host concatenates shards.

Bin counts (K1, K2) are the max per-half in-degree over all windows/cores,
computed from the input (compile cache keyed on them).
"""
import numpy as np
import concourse.bass as bass
import concourse.bacc as bacc
import concourse.tile as tile
from concourse import mybir
from concourse import library_config
from concourse.bass import ds
from concourse.bass_utils import run_bass_kernel_spmd

P = 128
F32 = mybir.dt.float32
I16 = mybir.dt.int16
AF = mybir.ActivationFunctionType
OP = mybir.AluOpType

N_NODES = 50000
NC = 8
SHARD = N_NODES // NC          # 6250
NW = (SHARD + P - 1) // P      # 49 windows
TAILR = SHARD - (NW - 1) * P   # 106 rows in the last window
HALF = 32768                   # int16 index limit for dma_gather
MASKVAL = -300.0
W1O, W2O = 80, 42              # used cols of the two tables
T1W, T2W = 128, 64             # table row widths (512B / 256B)


def _fold_params(W1, a1_src, a1_dst, W2, a2_src, a2_dst):
    def fold(W, a):
        heads, od = a.shape
        return np.einsum("cho,ho->ch", W.reshape(W.shape[0], heads, od), a)
    W_ext1 = np.concatenate([W1, fold(W1, a1_src), fold(W1, a1_dst)], axis=1)
    W_ext2 = np.concatenate([W2, fold(W2, a2_src), fold(W2, a2_dst)], axis=1)
    return (np.ascontiguousarray(W_ext1, np.float32),
            np.ascontiguousarray(W_ext2, np.float32))


def _core_edges(src, dst, c):
    """Edges (incl. self loops) for core c: returns (esrc, eld)."""
    lo = c * SHARD
    m = (dst >= lo) & (dst < lo + SHARD)
    esrc = src[m]
    eld = dst[m] - lo
    selfn = np.arange(lo, lo + SHARD, dtype=np.int64)
    esrc = np.concatenate([esrc, selfn])
    eld = np.concatenate([eld, selfn - lo])
    return esrc, eld


def required_T(edge_index, N=None):
    """Global (K1, K2): max lo/hi bin count over all cores' windows."""
    src = np.asarray(edge_index[0]).astype(np.int64)
    dst = np.asarray(edge_index[1]).astype(np.int64)
    K1 = K2 = 0
    for c in range(NC):
        esrc, eld = _core_edges(src, dst, c)
        hi = (esrc >= HALF).astype(np.int64)
        cnt = np.bincount(eld * 2 + hi, minlength=SHARD * 2).reshape(SHARD, 2)
        K1 = max(K1, int(cnt[:, 0].max()))
        K2 = max(K2, int(cnt[:, 1].max()))
    return (K1, K2)


def _gather_splits(K):
    """Split K*128 slots into dma_gather calls of <=1024 idxs (8 chunks)."""
    out = []
    left = K
    while left > 0:
        take = min(8, left)
        out.append(take * P)
        left -= take
    return out


def _wrap_blocks(flat, nidx_list):
    """Concat per-sub-block wrapped int16 idx layouts -> [128, total//16]."""
    cols = []
    off = 0
    for n in nidx_list:
        blk = flat[off:off + n]
        w = blk.reshape(-1, 16).T  # [16, n//16]
        cols.append(np.tile(w, (8, 1)))
        off += n
    return np.concatenate(cols, axis=1).astype(np.int16)


def _prep_edges(src, dst, K1, K2):
    """Per-core (idx_dram [128, NW*K*8] i16, mask_dram [128, NW*K] f32)."""
    K = K1 + K2
    splits = _gather_splits(K1) + _gather_splits(K2)
    per_core = []
    for c in range(NC):
        esrc, eld = _core_edges(src, dst, c)
        hi = (esrc >= HALF).astype(np.int64)
        key = eld * 2 + hi
        order = np.argsort(key, kind="stable")
        ks = key[order]
        starts = np.searchsorted(ks, np.arange(SHARD * 2))
        rank = np.arange(len(ks)) - starts[ks]
        e_src = esrc[order]
        e_ld = eld[order]
        e_hi = hi[order]
        w = e_ld >> 7
        d = e_ld & 127
        k = np.where(e_hi == 0, rank, K1 + rank)
        val = np.where(e_hi == 0, e_src, e_src - HALF)
        bins = np.zeros((NW, K, P), np.int64)
        mask = np.full((NW, P, K), MASKVAL, np.float32)
        bins[w, k, d] = val
        mask[w, d, k] = 0.0
        idx_cols = [_wrap_blocks(bins[wi].reshape(-1), splits)
                    for wi in range(NW)]
        idx_dram = np.ascontiguousarray(np.concatenate(idx_cols, axis=1))
        mask_dram = np.ascontiguousarray(
            mask.transpose(1, 0, 2).reshape(P, NW * K))
        per_core.append((idx_dram, mask_dram))
    return per_core


def build_kernel(K1, K2, reps=1):
    K = K1 + K2
    ICOL = K * 8          # idx cols per window
    splits1 = _gather_splits(K1)
    splits2 = _gather_splits(K2)

    nc = bacc.Bacc("TRN2", target_bir_lowering=False, debug=False,
                   num_swdge_queues=1)

    xT = nc.dram_tensor("xT", [P, SHARD], F32, kind="ExternalInput")
    W_ext1 = nc.dram_tensor("W_ext1", [P, W1O], F32, kind="ExternalInput")
    W_ext2 = nc.dram_tensor("W_ext2", [64, W2O], F32, kind="ExternalInput")
    b1m = nc.dram_tensor("b1m", [P, 64], F32, kind="ExternalInput")
    b2m = nc.dram_tensor("b2m", [P, 40], F32, kind="ExternalInput")
    ident_in = nc.dram_tensor("ident_in", [P, P], F32, kind="ExternalInput")
    idx_in = nc.dram_tensor("idx_in", [P, NW * ICOL], I16, kind="ExternalInput")
    mask_in = nc.dram_tensor("mask_in", [P, NW * K], F32, kind="ExternalInput")
    out = nc.dram_tensor("out", [SHARD, 40], F32, kind="ExternalOutput")

    t1_shard = nc.dram_tensor("t1_shard", [SHARD, T1W], F32)
    t2_shard = nc.dram_tensor("t2_shard", [SHARD, T2W], F32)
    h1_dram = nc.dram_tensor("h1_dram", [SHARD, 64], F32)
    t1 = nc.dram_tensor("t1", [N_NODES, T1W], F32, addr_space="Shared")
    t2 = nc.dram_tensor("t2", [N_NODES, T2W], F32, addr_space="Shared")

    with tile.TileContext(nc) as tc:
        nc.gpsimd.load_library(library_config.mlp)
        cp = tc.alloc_tile_pool(name="const", bufs=1)
        w1_sb = cp.tile([P, W1O], F32)
        nc.sync.dma_start(out=w1_sb[:], in_=W_ext1[:])
        w2_sb = cp.tile([64, W2O], F32)
        nc.sync.dma_start(out=w2_sb[:], in_=W_ext2[:])
        b1_sb = cp.tile([P, 64], F32)
        nc.sync.dma_start(out=b1_sb[:], in_=b1m[:])
        b2_sb = cp.tile([P, 40], F32)
        nc.sync.dma_start(out=b2_sb[:], in_=b2m[:])
        ident_sb = cp.tile([P, P], F32)
        nc.sync.dma_start(out=ident_sb[:], in_=ident_in[:])

        # working tiles (fixed addresses, reused across loop iterations)
        wp = tc.alloc_tile_pool(name="work", bufs=1)
        xc = wp.tile([P, P], F32)
        hb = wp.tile([P, W1O], F32)
        gi = wp.tile([P, ICOL], I16)
        mk = wp.tile([P, K], F32)
        adg = wp.tile([P, 16], F32)
        g = wp.tile([P, K * T1W], F32)
        ee = wp.tile([P, K, 8], F32)
        ex = wp.tile([P, K, 8], F32)
        msg = wp.tile([P, K, 64], F32)
        U = wp.tile([P, 64], F32)
        den = wp.tile([P, 8], F32)
        rec = wp.tile([P, 8], F32)
        agg = wp.tile([P, 64], F32)
        em = wp.tile([P, 64], F32)
        h1 = wp.tile([P, 64], F32)
        h1c = wp.tile([P, 64], F32)
        hT = wp.tile([64, P], F32)
        h2b = wp.tile([P, W2O], F32)
        ob = wp.tile([P, 40], F32)
        pp = tc.alloc_tile_pool(name="ps", bufs=1, space="PSUM")
        ps1 = pp.tile([P, W1O], F32, space="PSUM")
        psT = pp.tile([64, P], F32, space="PSUM")
        ps2 = pp.tile([P, W2O], F32, space="PSUM")

        def a1_body(i_col, i_row, rows):
            nc.sync.dma_start(out=xc[:, 0:rows], in_=xT[:, i_col])
            nc.tensor.matmul(out=ps1[0:rows, :], lhsT=xc[:, 0:rows],
                             rhs=w1_sb[:], start=True, stop=True)
            nc.scalar.activation(out=hb[0:rows, :], in_=ps1[0:rows, :],
                                 func=AF.Copy)
            nc.sync.dma_start(out=t1_shard[i_row, 0:W1O], in_=hb[0:rows, :])

        def edge_body(i_row, i_idx, i_mask, rows, table, adtab, tshape, usedw,
                      NH, OD, post):
            """One window of the edge phase. i_* are ds() slices."""
            HC = NH * OD
            acol = usedw - 2 * NH
            gw = g[:].rearrange("p (k w) -> p k w", w=tshape)[:, 0:K, :]
            nc.sync.dma_start(out=gi[:], in_=idx_in[:, i_idx])
            nc.sync.dma_start(out=mk[:], in_=mask_in[:, i_mask])
            nc.sync.dma_start(out=adg[0:rows, 0:2 * NH],
                              in_=adtab[i_row, acol:acol + 2 * NH])
            off = 0
            coloff = 0
            for base, n_list in ((0, splits1), (1, splits2)):
                tab_ap = table[0:HALF, :] if base == 0 else table[HALF:N_NODES, :]
                for n_idx in n_list:
                    nc.gpsimd.dma_gather(
                        out_ap=gw[:, off:off + n_idx // P, :],
                        in_ap=tab_ap,
                        idxs_ap=gi[:, coloff:coloff + n_idx // 16],
                        num_idxs=n_idx, num_idxs_reg=n_idx, elem_size=tshape,
                        queue_num=0)
                    off += n_idx // P
                    coloff += n_idx // 16
            # logits: e = alpha_src[slot] + alpha_dst[d] + mask
            nc.vector.tensor_add(
                out=ee[:, :, 0:NH], in0=gw[:, :, acol:acol + NH],
                in1=adg[:, None, NH:2 * NH].to_broadcast([P, K, NH]))
            nc.vector.tensor_add(
                out=ee[:, :, 0:NH], in0=ee[:, :, 0:NH],
                in1=mk[:, :, None].to_broadcast([P, K, NH]))
            nc.vector.scalar_tensor_tensor(
                out=ee[:, :, 0:NH], in0=ee[:, :, 0:NH], scalar=0.2,
                in1=ee[:, :, 0:NH], op0=OP.mult, op1=OP.max)
            nc.scalar.activation(out=ex[:, :, 0:NH], in_=ee[:, :, 0:NH],
                                 func=AF.Exp)
            # msg = h[slot] * ex ; U/den = reduce over k ; agg = U/den
            nc.vector.tensor_tensor(
                out=msg[:, :, 0:HC].rearrange("p k (h o) -> p k h o", o=OD),
                in0=gw[:, :, 0:HC].rearrange("p k (h o) -> p k h o", o=OD),
                in1=ex[:, :, 0:NH, None].to_broadcast([P, K, NH, OD]),
                op=OP.mult)
            nc.vector.reduce_sum(
                out=U[:, 0:HC, None],
                in_=msg[:, :, 0:HC].rearrange("p k f -> p f k"),
                axis=mybir.AxisListType.X)
            nc.vector.reduce_sum(
                out=den[:, 0:NH, None],
                in_=ex[:, :, 0:NH].rearrange("p k h -> p h k"),
                axis=mybir.AxisListType.X)
            nc.vector.reciprocal(rec[:, 0:NH], den[:, 0:NH])
            nc.vector.tensor_tensor(
                out=agg[:, 0:HC].rearrange("p (h o) -> p h o", o=OD),
                in0=U[:, 0:HC].rearrange("p (h o) -> p h o", o=OD),
                in1=rec[:, 0:NH, None].to_broadcast([P, NH, OD]), op=OP.mult)
            post(rows)

        def post1(i_h1):
            def post(rows):
                nc.vector.tensor_add(out=agg[:, 0:64], in0=agg[:, 0:64],
                                     in1=b1_sb[:])
                nc.scalar.activation(out=em[:], in_=agg[:, 0:64], func=AF.Exp)
                nc.vector.tensor_scalar(out=em[:], in0=em[:], scalar1=-1.0,
                                        scalar2=0.0, op0=OP.add, op1=OP.min)
                nc.vector.scalar_tensor_tensor(
                    out=h1[:], in0=agg[:, 0:64], scalar=0.0, in1=em[:],
                    op0=OP.max, op1=OP.add)
                nc.sync.dma_start(out=h1_dram[i_h1, :], in_=h1[0:rows, :])
            return post

        def post2(i_out):
            def post(rows):
                nc.vector.tensor_add(out=agg[:, 0:40], in0=agg[:, 0:40],
                                     in1=b2_sb[:])
                nc.vector.reduce_max(out=den[:, 0:1, None],
                                     in_=agg[:, None, 0:40],
                                     axis=mybir.AxisListType.X)
                nc.vector.tensor_sub(out=em[:, 0:40], in0=agg[:, 0:40],
                                     in1=den[:, 0:1].to_broadcast([P, 40]))
                nc.scalar.activation(out=ob[:], in_=em[:, 0:40], func=AF.Exp)
                nc.vector.reduce_sum(out=den[:, 1:2, None], in_=ob[:, None, :],
                                     axis=mybir.AxisListType.X)
                nc.scalar.activation(out=rec[:, 0:1], in_=den[:, 1:2],
                                     func=AF.Ln)
                nc.vector.tensor_sub(out=ob[:], in0=em[:, 0:40],
                                     in1=rec[:, 0:1].to_broadcast([P, 40]))
                nc.sync.dma_start(out=out[i_out, :], in_=ob[0:rows, :])
            return post

        def a2_body(i_row, rows):
            nc.sync.dma_start(out=h1c[0:rows, :], in_=h1_dram[i_row, :])
            nc.tensor.transpose(out=psT[:], in_=h1c[:], identity=ident_sb[:])
            nc.scalar.activation(out=hT[:], in_=psT[:], func=AF.Copy)
            nc.tensor.matmul(out=ps2[:], lhsT=hT[:], rhs=w2_sb[:],
                             start=True, stop=True)
            nc.scalar.activation(out=h2b[:], in_=ps2[:], func=AF.Copy)
            nc.sync.dma_start(out=t2_shard[i_row, 0:W2O], in_=h2b[0:rows, :])

        TB = (NW - 1) * P  # tail base row
        for rep in range(reps):
            # ---- A1 ----
            with tc.For_i(0, TB, P) as i:
                a1_body(ds(i, P), ds(i, P), P)
            a1_body(ds(TB, TAILR), ds(TB, TAILR), TAILR)
            nc.gpsimd.collective_compute(
                "AllGather", OP.bypass, replica_groups=[list(range(NC))],
                ins=[t1_shard[:]], outs=[t1[:]])

            # ---- B1 ----
            with tc.For_i(0, NW - 1) as i:
                edge_body(ds(i * P, P), ds(i * ICOL, ICOL), ds(i * K, K),
                          P, t1, t1_shard, T1W, W1O, 8, 8, post1(ds(i * P, P)))
            edge_body(ds(TB, TAILR), ds((NW - 1) * ICOL, ICOL),
                      ds((NW - 1) * K, K), TAILR, t1, t1_shard, T1W, W1O,
                      8, 8, post1(ds(TB, TAILR)))

            # ---- A2 ----
            with tc.For_i(0, TB, P) as i:
                a2_body(ds(i, P), P)
            a2_body(ds(TB, TAILR), TAILR)
            nc.gpsimd.collective_compute(
                "AllGather", OP.bypass, replica_groups=[list(range(NC))],
                ins=[t2_shard[:]], outs=[t2[:]])

            # ---- B2 ----
            with tc.For_i(0, NW - 1) as i:
                edge_body(ds(i * P, P), ds(i * ICOL, ICOL), ds(i * K, K),
                          P, t2, t2_shard, T2W, W2O, 1, 40, post2(ds(i * P, P)))
            edge_body(ds(TB, TAILR), ds((NW - 1) * ICOL, ICOL),
                      ds((NW - 1) * K, K), TAILR, t2, t2_shard, T2W, W2O,
                      1, 40, post2(ds(TB, TAILR)))

        pp.release()
        wp.release()
        cp.release()

    nc.compile()
    return nc


_CACHE = {}


def _get_nc(T, reps=1):
    key = (T, reps)
    if key not in _CACHE:
        K1, K2 = T
        _CACHE[key] = build_kernel(K1, K2, reps=reps)
    return _CACHE[key]


def make_in_maps(x, edge_index, W1, a1_src, a1_dst, b1, W2, a2_src, a2_dst, b2,
                 T, N=None):
    K1, K2 = T
    W_ext1, W_ext2 = _fold_params(W1, a1_src, a1_dst, W2, a2_src, a2_dst)
    src = np.asarray(edge_index[0]).astype(np.int64)
    dst = np.asarray(edge_index[1]).astype(np.int64)
    per_core = _prep_edges(src, dst, K1, K2)
    xTf = np.ascontiguousarray(np.asarray(x, np.float32).T)
    shared = {
        "W_ext1": W_ext1, "W_ext2": W_ext2,
        "b1m": np.tile(np.asarray(b1, np.float32)[None, :], (P, 1)),
        "b2m": np.tile(np.asarray(b2, np.float32)[None, :], (P, 1)),
        "ident_in": np.eye(P, dtype=np.float32),
    }
    return [dict(shared, idx_in=ix, mask_in=mk,
                 xT=np.ascontiguousarray(xTf[:, c * SHARD:(c + 1) * SHARD]))
            for c, (ix, mk) in enumerate(per_core)]


def kernel(x, edge_index, W1, a1_src, a1_dst, b1, W2, a2_src, a2_dst, b2,
           reps=1, nc_override=None):
    x = np.asarray(x, np.float32)
    edge_index = np.asarray(edge_index)
    args = [np.asarray(a, np.float32) for a in
            (W1, a1_src, a1_dst, b1, W2, a2_src, a2_dst, b2)]
    T = required_T(edge_index)
    in_maps = make_in_maps(x, edge_index, *args, T)
    nc = nc_override if nc_override is not None else _get_nc(T, reps)
    res = run_bass_kernel_spmd(nc, in_maps, list(range(NC)))
    return np.concatenate([res.results[c]["out"] for c in range(NC)], axis=0)


# revision 9
# speedup vs baseline: 30.3721x; 1.3916x over previous
"""2-layer GAT (nn_GATNet) on 8 TRN2 NeuronCores — self-contained kernel.

Design (SPMD, one program on 8 cores, dst-node sharding 6250/core).

The runtime here charges ~40-75us per STATIC instruction, so the kernel is
built around hardware For_i loops with tiny bodies (~150 static instructions
total) and a matmul-free edge phase:

  A1 (For_i over 49 windows): t1_shard = x @ [W1 | W1 a1_src | W1 a1_dst]
     for the local dst shard ([6250,128] rows, 80 cols used, 512B rows),
     then AllGather -> t1 [50000,128] on every core.
  B1 (For_i over windows): K-binned dst-major edge layout. Each window = 128
     dst nodes (one per partition); bin k of dst d sits at gather slot
     k*128+d, so dma_gather (non-transpose) lands each dst's K bins on its
     own partition: g[d, k, :]. Bins are split into a lo section (src <
     32768, K1 bins) and a hi section (src-32768, K2 bins) because dma_gather
     indices are int16; hi gathers use a base-shifted table AP. Self loops
     are ordinary bins; pad bins gather row 0 and carry a -300 logit mask.
     Per window: ~7 dma_gathers (<=1024 idxs each, the ucode ring cap) +
     ~14 DVE/Act ops: logits = g[:,:,acol:acol+NH] + alpha_dst (free-dim
     broadcast from a direct window DMA of the local shard rows) + mask;
     ex = exp(leakyrelu); msg = g[:,:,0:HC]*ex; U/denom = free-axis reduce
     over k; agg = U/denom. No PE, no one-hot, no transposes.
  A2 (For_i): t2_shard = elu(h1) @ [W2 | ...] ([6250,64] rows, 42 used,
     256B rows), AllGather -> t2.
  B2: same edge phase with 1 head / 40 dims + log_softmax, writes the local
     [6250, 40] output shard; host concatenates shards.

Bin counts (K1, K2) are the max per-half in-degree over all windows/cores,
computed from the input (compile cache keyed on them).
"""
import numpy as np
import concourse.bass as bass
import concourse.bacc as bacc
import concourse.tile as tile
from concourse import mybir
from concourse import library_config
from concourse.bass import ds
from concourse.bass_utils import run_bass_kernel_spmd

P = 128
F32 = mybir.dt.float32
I16 = mybir.dt.int16
AF = mybir.ActivationFunctionType
OP = mybir.AluOpType

N_NODES = 50000
NC = 8
SHARD = N_NODES // NC          # 6250
NW = (SHARD + P - 1) // P      # 49 windows
TAILR = SHARD - (NW - 1) * P   # 106 rows in the last window
HALF = 32768                   # int16 index limit for dma_gather
MASKVAL = -300.0
W1O, W2O = 80, 42              # used cols of the two tables
T1W, T2W = 128, 64             # table row widths (512B / 256B)


def _fold_params(W1, a1_src, a1_dst, W2, a2_src, a2_dst):
    def fold(W, a):
        heads, od = a.shape
        return np.einsum("cho,ho->ch", W.reshape(W.shape[0], heads, od), a)
    W_ext1 = np.concatenate([W1, fold(W1, a1_src), fold(W1, a1_dst)], axis=1)
    W_ext2 = np.concatenate([W2, fold(W2, a2_src), fold(W2, a2_dst)], axis=1)
    return (np.ascontiguousarray(W_ext1, np.float32),
            np.ascontiguousarray(W_ext2, np.float32))


def _core_edges(src, dst, c):
    """Edges (incl. self loops) for core c: returns (esrc, eld)."""
    lo = c * SHARD
    m = (dst >= lo) & (dst < lo + SHARD)
    esrc = src[m]
    eld = dst[m] - lo
    selfn = np.arange(lo, lo + SHARD, dtype=np.int64)
    esrc = np.concatenate([esrc, selfn])
    eld = np.concatenate([eld, selfn - lo])
    return esrc, eld


def required_T(edge_index, N=None):
    """Global (K1, K2): max lo/hi bin count over all cores' windows."""
    src = np.asarray(edge_index[0]).astype(np.int64)
    dst = np.asarray(edge_index[1]).astype(np.int64)
    K1 = K2 = 0
    for c in range(NC):
        esrc, eld = _core_edges(src, dst, c)
        hi = (esrc >= HALF).astype(np.int64)
        cnt = np.bincount(eld * 2 + hi, minlength=SHARD * 2).reshape(SHARD, 2)
        K1 = max(K1, int(cnt[:, 0].max()))
        K2 = max(K2, int(cnt[:, 1].max()))
    return (K1, K2)


def _gather_splits(K):
    """Split K*128 slots into dma_gather calls of <=1024 idxs (8 chunks)."""
    out = []
    left = K
    while left > 0:
        take = min(8, left)
        out.append(take * P)
        left -= take
    return out


def _wrap_blocks(flat, nidx_list):
    """Concat per-sub-block wrapped int16 idx layouts -> [128, total//16]."""
    cols = []
    off = 0
    for n in nidx_list:
        blk = flat[off:off + n]
        w = blk.reshape(-1, 16).T  # [16, n//16]
        cols.append(np.tile(w, (8, 1)))
        off += n
    return np.concatenate(cols, axis=1).astype(np.int16)


def _prep_edges(src, dst, K1, K2):
    """Per-core (idx_dram [128, NW*K*8] i16, mask_dram [128, NW*K] f32)."""
    K = K1 + K2
    splits = _gather_splits(K1) + _gather_splits(K2)
    per_core = []
    for c in range(NC):
        esrc, eld = _core_edges(src, dst, c)
        hi = (esrc >= HALF).astype(np.int64)
        key = eld * 2 + hi
        order = np.argsort(key, kind="stable")
        ks = key[order]
        starts = np.searchsorted(ks, np.arange(SHARD * 2))
        rank = np.arange(len(ks)) - starts[ks]
        e_src = esrc[order]
        e_ld = eld[order]
        e_hi = hi[order]
        w = e_ld >> 7
        d = e_ld & 127
        k = np.where(e_hi == 0, rank, K1 + rank)
        val = np.where(e_hi == 0, e_src, e_src - HALF)
        bins = np.zeros((NW, K, P), np.int64)
        mask = np.full((NW, P, K), MASKVAL, np.float32)
        bins[w, k, d] = val
        mask[w, d, k] = 0.0
        idx_cols = [_wrap_blocks(bins[wi].reshape(-1), splits)
                    for wi in range(NW)]
        idx_dram = np.ascontiguousarray(np.concatenate(idx_cols, axis=1))
        mask_dram = np.ascontiguousarray(
            mask.transpose(1, 0, 2).reshape(P, NW * K))
        per_core.append((idx_dram, mask_dram))
    return per_core


def build_kernel(K1, K2, reps=1):
    K = K1 + K2
    ICOL = K * 8          # idx cols per window
    splits1 = _gather_splits(K1)
    splits2 = _gather_splits(K2)

    nc = bacc.Bacc("TRN2", target_bir_lowering=False, debug=False,
                   num_swdge_queues=1)

    xT = nc.dram_tensor("xT", [P, SHARD], F32, kind="ExternalInput")
    W_ext1 = nc.dram_tensor("W_ext1", [P, W1O], F32, kind="ExternalInput")
    W_ext2 = nc.dram_tensor("W_ext2", [64, W2O], F32, kind="ExternalInput")
    b1m = nc.dram_tensor("b1m", [P, 64], F32, kind="ExternalInput")
    b2m = nc.dram_tensor("b2m", [P, 40], F32, kind="ExternalInput")
    ident_in = nc.dram_tensor("ident_in", [P, P], F32, kind="ExternalInput")
    idx_in = nc.dram_tensor("idx_in", [P, NW * ICOL], I16, kind="ExternalInput")
    mask_in = nc.dram_tensor("mask_in", [P, NW * K], F32, kind="ExternalInput")
    out = nc.dram_tensor("out", [SHARD, 40], F32, kind="ExternalOutput")

    t1_shard = nc.dram_tensor("t1_shard", [SHARD, T1W], F32)
    t2_shard = nc.dram_tensor("t2_shard", [SHARD, T2W], F32)
    h1_dram = nc.dram_tensor("h1_dram", [SHARD, 64], F32)
    t1 = nc.dram_tensor("t1", [N_NODES, T1W], F32, addr_space="Shared")
    t2 = nc.dram_tensor("t2", [N_NODES, T2W], F32, addr_space="Shared")

    with tile.TileContext(nc) as tc:
        nc.gpsimd.load_library(library_config.mlp)
        cp = tc.alloc_tile_pool(name="const", bufs=1)
        w1_sb = cp.tile([P, W1O], F32)
        nc.sync.dma_start(out=w1_sb[:], in_=W_ext1[:])
        w2_sb = cp.tile([64, W2O], F32)
        nc.sync.dma_start(out=w2_sb[:], in_=W_ext2[:])
        b1_sb = cp.tile([P, 64], F32)
        nc.sync.dma_start(out=b1_sb[:], in_=b1m[:])
        b2_sb = cp.tile([P, 40], F32)
        nc.sync.dma_start(out=b2_sb[:], in_=b2m[:])
        ident_sb = cp.tile([P, P], F32)
        nc.sync.dma_start(out=ident_sb[:], in_=ident_in[:])

        # two tile sets (double-buffering across loop iterations: set 1's
        # gathers/DMAs overlap set 0's compute, so cross-engine waits are
        # usually pre-satisfied when reached)
        wp = tc.alloc_tile_pool(name="work", bufs=1)
        pp = tc.alloc_tile_pool(name="ps", bufs=1, space="PSUM")
        S = []
        for s in range(2):
            t = {}
            t["xc"] = wp.tile([P, P], F32, name=f"xc{s}")
            t["hb"] = wp.tile([P, W1O], F32, name=f"hb{s}")
            t["gi"] = wp.tile([P, ICOL], I16, name=f"gi{s}")
            t["mk"] = wp.tile([P, K], F32, name=f"mk{s}")
            t["adg"] = wp.tile([P, 16], F32, name=f"adg{s}")
            t["g"] = wp.tile([P, K * T1W], F32, name=f"g{s}")
            t["ee"] = wp.tile([P, K, 8], F32, name=f"ee{s}")
            t["ex"] = wp.tile([P, K, 8], F32, name=f"ex{s}")
            t["msg"] = wp.tile([P, K, 64], F32, name=f"msg{s}")
            t["U"] = wp.tile([P, 64], F32, name=f"U{s}")
            t["den"] = wp.tile([P, 8], F32, name=f"den{s}")
            t["rec"] = wp.tile([P, 8], F32, name=f"rec{s}")
            t["agg"] = wp.tile([P, 64], F32, name=f"agg{s}")
            t["em"] = wp.tile([P, 64], F32, name=f"em{s}")
            t["h1"] = wp.tile([P, 64], F32, name=f"h1_{s}")
            t["h1c"] = wp.tile([P, 64], F32, name=f"h1c{s}")
            t["hT"] = wp.tile([64, P], F32, name=f"hT{s}")
            t["h2b"] = wp.tile([P, W2O], F32, name=f"h2b{s}")
            t["ob"] = wp.tile([P, 40], F32, name=f"ob{s}")
            t["ps1"] = pp.tile([P, W1O], F32, space="PSUM", name=f"ps1_{s}")
            t["psT"] = pp.tile([64, P], F32, space="PSUM", name=f"psT{s}")
            t["ps2"] = pp.tile([P, W2O], F32, space="PSUM", name=f"ps2_{s}")
            S.append(t)

        def a1_body(t, i_col, i_row, rows):
            nc.sync.dma_start(out=t["xc"][:, 0:rows], in_=xT[:, i_col])
            nc.tensor.matmul(out=t["ps1"][0:rows, :], lhsT=t["xc"][:, 0:rows],
                             rhs=w1_sb[:], start=True, stop=True)
            nc.scalar.activation(out=t["hb"][0:rows, :], in_=t["ps1"][0:rows, :],
                                 func=AF.Copy)
            nc.sync.dma_start(out=t1_shard[i_row, 0:W1O], in_=t["hb"][0:rows, :])

        def edge_body(t, i_row, i_idx, i_mask, rows, table, adtab, tshape,
                      usedw, NH, OD, post):
            """One window of the edge phase. i_* are ds() slices."""
            HC = NH * OD
            acol = usedw - 2 * NH
            gi, mk, adg = t["gi"], t["mk"], t["adg"]
            ee, ex, msg = t["ee"], t["ex"], t["msg"]
            U, den, rec, agg = t["U"], t["den"], t["rec"], t["agg"]
            gw = t["g"][:].rearrange("p (k w) -> p k w", w=tshape)[:, 0:K, :]
            nc.sync.dma_start(out=gi[:], in_=idx_in[:, i_idx])
            nc.sync.dma_start(out=mk[:], in_=mask_in[:, i_mask])
            nc.sync.dma_start(out=adg[0:rows, 0:2 * NH],
                              in_=adtab[i_row, acol:acol + 2 * NH])
            off = 0
            coloff = 0
            for base, n_list in ((0, splits1), (1, splits2)):
                tab_ap = table[0:HALF, :] if base == 0 else table[HALF:N_NODES, :]
                for n_idx in n_list:
                    nc.gpsimd.dma_gather(
                        out_ap=gw[:, off:off + n_idx // P, :],
                        in_ap=tab_ap,
                        idxs_ap=gi[:, coloff:coloff + n_idx // 16],
                        num_idxs=n_idx, num_idxs_reg=n_idx, elem_size=tshape,
                        queue_num=0)
                    off += n_idx // P
                    coloff += n_idx // 16
            # logits: e = alpha_src[slot] + alpha_dst[d] + mask
            nc.vector.tensor_add(
                out=ee[:, :, 0:NH], in0=gw[:, :, acol:acol + NH],
                in1=adg[:, None, NH:2 * NH].to_broadcast([P, K, NH]))
            nc.vector.tensor_add(
                out=ee[:, :, 0:NH], in0=ee[:, :, 0:NH],
                in1=mk[:, :, None].to_broadcast([P, K, NH]))
            nc.vector.scalar_tensor_tensor(
                out=ee[:, :, 0:NH], in0=ee[:, :, 0:NH], scalar=0.2,
                in1=ee[:, :, 0:NH], op0=OP.mult, op1=OP.max)
            nc.scalar.activation(out=ex[:, :, 0:NH], in_=ee[:, :, 0:NH],
                                 func=AF.Exp)
            # msg = h[slot] * ex ; U/den = reduce over k ; agg = U/den
            nc.vector.tensor_tensor(
                out=msg[:, :, 0:HC].rearrange("p k (h o) -> p k h o", o=OD),
                in0=gw[:, :, 0:HC].rearrange("p k (h o) -> p k h o", o=OD),
                in1=ex[:, :, 0:NH, None].to_broadcast([P, K, NH, OD]),
                op=OP.mult)
            nc.vector.reduce_sum(
                out=U[:, 0:HC, None],
                in_=msg[:, :, 0:HC].rearrange("p k f -> p f k"),
                axis=mybir.AxisListType.X)
            nc.vector.reduce_sum(
                out=den[:, 0:NH, None],
                in_=ex[:, :, 0:NH].rearrange("p k h -> p h k"),
                axis=mybir.AxisListType.X)
            nc.vector.reciprocal(rec[:, 0:NH], den[:, 0:NH])
            nc.vector.tensor_tensor(
                out=agg[:, 0:HC].rearrange("p (h o) -> p h o", o=OD),
                in0=U[:, 0:HC].rearrange("p (h o) -> p h o", o=OD),
                in1=rec[:, 0:NH, None].to_broadcast([P, NH, OD]), op=OP.mult)
            post(t, rows)

        def post1(i_h1):
            def post(t, rows):
                agg, em, h1 = t["agg"], t["em"], t["h1"]
                nc.vector.tensor_add(out=agg[:, 0:64], in0=agg[:, 0:64],
                                     in1=b1_sb[:])
                nc.scalar.activation(out=em[:], in_=agg[:, 0:64], func=AF.Exp)
                nc.vector.tensor_scalar(out=em[:], in0=em[:], scalar1=-1.0,
                                        scalar2=0.0, op0=OP.add, op1=OP.min)
                nc.vector.scalar_tensor_tensor(
                    out=h1[:], in0=agg[:, 0:64], scalar=0.0, in1=em[:],
                    op0=OP.max, op1=OP.add)
                nc.sync.dma_start(out=h1_dram[i_h1, :], in_=h1[0:rows, :])
            return post

        def post2(i_out):
            def post(t, rows):
                agg, em, ob = t["agg"], t["em"], t["ob"]
                den, rec = t["den"], t["rec"]
                nc.vector.tensor_add(out=agg[:, 0:40], in0=agg[:, 0:40],
                                     in1=b2_sb[:])
                nc.vector.reduce_max(out=den[:, 0:1, None],
                                     in_=agg[:, None, 0:40],
                                     axis=mybir.AxisListType.X)
                nc.vector.tensor_sub(out=em[:, 0:40], in0=agg[:, 0:40],
                                     in1=den[:, 0:1].to_broadcast([P, 40]))
                nc.scalar.activation(out=ob[:], in_=em[:, 0:40], func=AF.Exp)
                nc.vector.reduce_sum(out=den[:, 1:2, None], in_=ob[:, None, :],
                                     axis=mybir.AxisListType.X)
                nc.scalar.activation(out=rec[:, 0:1], in_=den[:, 1:2],
                                     func=AF.Ln)
                nc.vector.tensor_sub(out=ob[:], in0=em[:, 0:40],
                                     in1=rec[:, 0:1].to_broadcast([P, 40]))
                nc.sync.dma_start(out=out[i_out, :], in_=ob[0:rows, :])
            return post

        def a2_body(t, i_row, rows):
            nc.sync.dma_start(out=t["h1c"][0:rows, :], in_=h1_dram[i_row, :])
            nc.tensor.transpose(out=t["psT"][:], in_=t["h1c"][:],
                                identity=ident_sb[:])
            nc.scalar.activation(out=t["hT"][:], in_=t["psT"][:], func=AF.Copy)
            nc.tensor.matmul(out=t["ps2"][:], lhsT=t["hT"][:], rhs=w2_sb[:],
                             start=True, stop=True)
            nc.scalar.activation(out=t["h2b"][:], in_=t["ps2"][:], func=AF.Copy)
            nc.sync.dma_start(out=t2_shard[i_row, 0:W2O], in_=t["h2b"][0:rows, :])

        TB = (NW - 1) * P  # tail base row (windows 0..47 paired in loops)
        for rep in range(reps):
            # ---- A1 ----
            with tc.For_i(0, TB, 2 * P) as i:
                a1_body(S[0], ds(i, P), ds(i, P), P)
                a1_body(S[1], ds(i + P, P), ds(i + P, P), P)
            a1_body(S[0], ds(TB, TAILR), ds(TB, TAILR), TAILR)
            nc.gpsimd.collective_compute(
                "AllGather", OP.bypass, replica_groups=[list(range(NC))],
                ins=[t1_shard[:]], outs=[t1[:]])

            # ---- B1 ----
            with tc.For_i(0, NW - 1, 2) as i:
                edge_body(S[0], ds(i * P, P), ds(i * ICOL, ICOL), ds(i * K, K),
                          P, t1, t1_shard, T1W, W1O, 8, 8, post1(ds(i * P, P)))
                edge_body(S[1], ds(i * P + P, P), ds(i * ICOL + ICOL, ICOL),
                          ds(i * K + K, K), P, t1, t1_shard, T1W, W1O, 8, 8,
                          post1(ds(i * P + P, P)))
            edge_body(S[0], ds(TB, TAILR), ds((NW - 1) * ICOL, ICOL),
                      ds((NW - 1) * K, K), TAILR, t1, t1_shard, T1W, W1O,
                      8, 8, post1(ds(TB, TAILR)))

            # ---- A2 ----
            with tc.For_i(0, TB, 2 * P) as i:
                a2_body(S[0], ds(i, P), P)
                a2_body(S[1], ds(i + P, P), P)
            a2_body(S[0], ds(TB, TAILR), TAILR)
            nc.gpsimd.collective_compute(
                "AllGather", OP.bypass, replica_groups=[list(range(NC))],
                ins=[t2_shard[:]], outs=[t2[:]])

            # ---- B2 ----
            with tc.For_i(0, NW - 1, 2) as i:
                edge_body(S[0], ds(i * P, P), ds(i * ICOL, ICOL), ds(i * K, K),
                          P, t2, t2_shard, T2W, W2O, 1, 40, post2(ds(i * P, P)))
                edge_body(S[1], ds(i * P + P, P), ds(i * ICOL + ICOL, ICOL),
                          ds(i * K + K, K), P, t2, t2_shard, T2W, W2O, 1, 40,
                          post2(ds(i * P + P, P)))
            edge_body(S[0], ds(TB, TAILR), ds((NW - 1) * ICOL, ICOL),
                      ds((NW - 1) * K, K), TAILR, t2, t2_shard, T2W, W2O,
                      1, 40, post2(ds(TB, TAILR)))

        pp.release()
        wp.release()
        cp.release()

    nc.compile()
    return nc


_CACHE = {}


def _get_nc(T, reps=1):
    key = (T, reps)
    if key not in _CACHE:
        K1, K2 = T
        _CACHE[key] = build_kernel(K1, K2, reps=reps)
    return _CACHE[key]


def make_in_maps(x, edge_index, W1, a1_src, a1_dst, b1, W2, a2_src, a2_dst, b2,
                 T, N=None):
    K1, K2 = T
    W_ext1, W_ext2 = _fold_params(W1, a1_src, a1_dst, W2, a2_src, a2_dst)
    src = np.asarray(edge_index[0]).astype(np.int64)
    dst = np.asarray(edge_index[1]).astype(np.int64)
    per_core = _prep_edges(src, dst, K1, K2)
    xTf = np.ascontiguousarray(np.asarray(x, np.float32).T)
    shared = {
        "W_ext1": W_ext1, "W_ext2": W_ext2,
        "b1m": np.tile(np.asarray(b1, np.float32)[None, :], (P, 1)),
        "b2m": np.tile(np.asarray(b2, np.float32)[None, :], (P, 1)),
        "ident_in": np.eye(P, dtype=np.float32),
    }
    return [dict(shared, idx_in=ix, mask_in=mk,
                 xT=np.ascontiguousarray(xTf[:, c * SHARD:(c + 1) * SHARD]))
            for c, (ix, mk) in enumerate(per_core)]


def kernel(x, edge_index, W1, a1_src, a1_dst, b1, W2, a2_src, a2_dst, b2,
           reps=1, nc_override=None):
    x = np.asarray(x, np.float32)
    edge_index = np.asarray(edge_index)
    args = [np.asarray(a, np.float32) for a in
            (W1, a1_src, a1_dst, b1, W2, a2_src, a2_dst, b2)]
    T = required_T(edge_index)
    in_maps = make_in_maps(x, edge_index, *args, T)
    nc = nc_override if nc_override is not None else _get_nc(T, reps)
    res = run_bass_kernel_spmd(nc, in_maps, list(range(NC)))
    return np.concatenate([res.results[c]["out"] for c in range(NC)], axis=0)


# revision 10
# speedup vs baseline: 54.3286x; 1.7888x over previous
"""2-layer GAT (nn_GATNet) on 8 TRN2 NeuronCores — self-contained kernel.

Design (SPMD, one program on 8 cores, dst-node sharding 6250/core).

The runtime here charges ~40-75us per STATIC instruction, so the kernel is
built around hardware For_i loops with tiny bodies (~150 static instructions
total) and a matmul-free edge phase:

  A1 (For_i over 49 windows): t1_shard = x @ [W1 | W1 a1_src | W1 a1_dst]
     for the local dst shard ([6250,128] rows, 80 cols used, 512B rows),
     then AllGather -> t1 [50000,128] on every core.
  B1 (For_i over windows): K-binned dst-major edge layout. Each window = 128
     dst nodes (one per partition); bin k of dst d sits at gather slot
     k*128+d, so dma_gather (non-transpose) lands each dst's K bins on its
     own partition: g[d, k, :]. Bins are split into a lo section (src <
     32768, K1 bins) and a hi section (src-32768, K2 bins) because dma_gather
     indices are int16; hi gathers use a base-shifted table AP. Self loops
     are ordinary bins; pad bins gather row 0 and carry a -300 logit mask.
     Per window: ~7 dma_gathers (<=1024 idxs each, the ucode ring cap) +
     ~14 DVE/Act ops: logits = g[:,:,acol:acol+NH] + alpha_dst (free-dim
     broadcast from a direct window DMA of the local shard rows) + mask;
     ex = exp(leakyrelu); msg = g[:,:,0:HC]*ex; U/denom = free-axis reduce
     over k; agg = U/denom. No PE, no one-hot, no transposes.
  A2 (For_i): t2_shard = elu(h1) @ [W2 | ...] ([6250,64] rows, 42 used,
     256B rows), AllGather -> t2.
  B2: same edge phase with 1 head / 40 dims + log_softmax, writes the local
     [6250, 40] output shard; host concatenates shards.

Bin counts (K1, K2) are the max per-half in-degree over all windows/cores,
computed from the input (compile cache keyed on them).
"""
import numpy as np
import concourse.bass as bass
import concourse.bacc as bacc
import concourse.tile as tile
from concourse import mybir
from concourse import library_config
from concourse.bass import ds
from concourse.bass_utils import run_bass_kernel_spmd

P = 128
F32 = mybir.dt.float32
I16 = mybir.dt.int16
AF = mybir.ActivationFunctionType
OP = mybir.AluOpType

N_NODES = 50000
NC = 8
SHARD = N_NODES // NC          # 6250
NW = (SHARD + P - 1) // P      # 49 windows
TAILR = SHARD - (NW - 1) * P   # 106 rows in the last window
HALF = 32768                   # int16 index limit for dma_gather
MASKVAL = -300.0
W1O, W2O = 80, 42              # used cols of the two tables
T1W, T2W = 128, 64             # table row widths (512B / 256B)


def _fold_params(W1, a1_src, a1_dst, W2, a2_src, a2_dst):
    def fold(W, a):
        heads, od = a.shape
        return np.einsum("cho,ho->ch", W.reshape(W.shape[0], heads, od), a)
    W_ext1 = np.concatenate([W1, fold(W1, a1_src), fold(W1, a1_dst)], axis=1)
    W_ext2 = np.concatenate([W2, fold(W2, a2_src), fold(W2, a2_dst)], axis=1)
    return (np.ascontiguousarray(W_ext1, np.float32),
            np.ascontiguousarray(W_ext2, np.float32))


def _core_edges(src, dst, c):
    """Edges (incl. self loops) for core c: returns (esrc, eld)."""
    lo = c * SHARD
    m = (dst >= lo) & (dst < lo + SHARD)
    esrc = src[m]
    eld = dst[m] - lo
    selfn = np.arange(lo, lo + SHARD, dtype=np.int64)
    esrc = np.concatenate([esrc, selfn])
    eld = np.concatenate([eld, selfn - lo])
    return esrc, eld


def required_T(edge_index, N=None):
    """Global (K1, K2): max lo/hi bin count over all cores' windows."""
    src = np.asarray(edge_index[0]).astype(np.int64)
    dst = np.asarray(edge_index[1]).astype(np.int64)
    K1 = K2 = 0
    for c in range(NC):
        esrc, eld = _core_edges(src, dst, c)
        hi = (esrc >= HALF).astype(np.int64)
        cnt = np.bincount(eld * 2 + hi, minlength=SHARD * 2).reshape(SHARD, 2)
        K1 = max(K1, int(cnt[:, 0].max()))
        K2 = max(K2, int(cnt[:, 1].max()))
    return (K1, K2)


def _gather_splits(K):
    """Split K*128 slots into dma_gather calls of <=1024 idxs (8 chunks)."""
    out = []
    left = K
    while left > 0:
        take = min(8, left)
        out.append(take * P)
        left -= take
    return out


def _wrap_blocks(flat, nidx_list):
    """Concat per-sub-block wrapped int16 idx layouts -> [128, total//16]."""
    cols = []
    off = 0
    for n in nidx_list:
        blk = flat[off:off + n]
        w = blk.reshape(-1, 16).T  # [16, n//16]
        cols.append(np.tile(w, (8, 1)))
        off += n
    return np.concatenate(cols, axis=1).astype(np.int16)


def _prep_edges(src, dst, K1, K2):
    """Per-core (idx_dram [128, NW*K*8] i16, mask_dram [128, NW*K] f32)."""
    K = K1 + K2
    splits = _gather_splits(K1) + _gather_splits(K2)
    per_core = []
    for c in range(NC):
        esrc, eld = _core_edges(src, dst, c)
        hi = (esrc >= HALF).astype(np.int64)
        key = eld * 2 + hi
        order = np.argsort(key, kind="stable")
        ks = key[order]
        starts = np.searchsorted(ks, np.arange(SHARD * 2))
        rank = np.arange(len(ks)) - starts[ks]
        e_src = esrc[order]
        e_ld = eld[order]
        e_hi = hi[order]
        w = e_ld >> 7
        d = e_ld & 127
        k = np.where(e_hi == 0, rank, K1 + rank)
        val = np.where(e_hi == 0, e_src, e_src - HALF)
        bins = np.zeros((NW, K, P), np.int64)
        mask = np.full((NW, P, K), MASKVAL, np.float32)
        bins[w, k, d] = val
        mask[w, d, k] = 0.0
        idx_cols = [_wrap_blocks(bins[wi].reshape(-1), splits)
                    for wi in range(NW)]
        idx_dram = np.ascontiguousarray(np.concatenate(idx_cols, axis=1))
        mask_dram = np.ascontiguousarray(
            mask.transpose(1, 0, 2).reshape(P, NW * K))
        per_core.append((idx_dram, mask_dram))
    return per_core


def build_kernel(K1, K2, reps=1):
    K = K1 + K2
    ICOL = K * 8          # idx cols per window
    splits1 = _gather_splits(K1)
    splits2 = _gather_splits(K2)

    nc = bacc.Bacc("TRN2", target_bir_lowering=False, debug=False,
                   num_swdge_queues=1)

    xT = nc.dram_tensor("xT", [P, SHARD], F32, kind="ExternalInput")
    W_ext1 = nc.dram_tensor("W_ext1", [P, W1O], F32, kind="ExternalInput")
    W_ext2 = nc.dram_tensor("W_ext2", [64, W2O], F32, kind="ExternalInput")
    b1m = nc.dram_tensor("b1m", [P, 64], F32, kind="ExternalInput")
    b2m = nc.dram_tensor("b2m", [P, 40], F32, kind="ExternalInput")
    ident_in = nc.dram_tensor("ident_in", [P, P], F32, kind="ExternalInput")
    idx_in = nc.dram_tensor("idx_in", [P, NW * ICOL], I16, kind="ExternalInput")
    mask_in = nc.dram_tensor("mask_in", [P, NW * K], F32, kind="ExternalInput")
    out = nc.dram_tensor("out", [SHARD, 40], F32, kind="ExternalOutput")

    t1_shard = nc.dram_tensor("t1_shard", [SHARD, T1W], F32)
    t2_shard = nc.dram_tensor("t2_shard", [SHARD, T2W], F32)
    t1 = nc.dram_tensor("t1", [N_NODES, T1W], F32, addr_space="Shared")
    t2 = nc.dram_tensor("t2", [N_NODES, T2W], F32, addr_space="Shared")

    with tile.TileContext(nc) as tc:
        nc.gpsimd.load_library(library_config.mlp)
        cp = tc.alloc_tile_pool(name="const", bufs=1)
        w1_sb = cp.tile([P, W1O], F32)
        nc.sync.dma_start(out=w1_sb[:], in_=W_ext1[:])
        w2_sb = cp.tile([64, W2O], F32)
        nc.sync.dma_start(out=w2_sb[:], in_=W_ext2[:])
        b1_sb = cp.tile([P, 64], F32)
        nc.sync.dma_start(out=b1_sb[:], in_=b1m[:])
        b2_sb = cp.tile([P, 40], F32)
        nc.sync.dma_start(out=b2_sb[:], in_=b2m[:])
        ident_sb = cp.tile([P, P], F32)
        nc.sync.dma_start(out=ident_sb[:], in_=ident_in[:])

        # two tile sets (double-buffering across loop iterations: set 1's
        # gathers/DMAs overlap set 0's compute, so cross-engine waits are
        # usually pre-satisfied when reached)
        wp = tc.alloc_tile_pool(name="work", bufs=1)
        pp = tc.alloc_tile_pool(name="ps", bufs=1, space="PSUM")
        S = []
        for s in range(2):
            t = {}
            t["xc"] = wp.tile([P, P], F32, name=f"xc{s}")
            t["hb"] = wp.tile([P, W1O], F32, name=f"hb{s}")
            t["gi"] = wp.tile([P, ICOL], I16, name=f"gi{s}")
            t["mk"] = wp.tile([P, K], F32, name=f"mk{s}")
            t["adg"] = wp.tile([P, 16], F32, name=f"adg{s}")
            t["g"] = wp.tile([P, K * T1W], F32, name=f"g{s}")
            t["ee"] = wp.tile([P, K, 8], F32, name=f"ee{s}")
            t["ex"] = wp.tile([P, K, 8], F32, name=f"ex{s}")
            t["msg"] = wp.tile([P, K, 64], F32, name=f"msg{s}")
            t["U"] = wp.tile([P, 64], F32, name=f"U{s}")
            t["den"] = wp.tile([P, 8], F32, name=f"den{s}")
            t["rec"] = wp.tile([P, 8], F32, name=f"rec{s}")
            t["agg"] = wp.tile([P, 64], F32, name=f"agg{s}")
            t["em"] = wp.tile([P, 64], F32, name=f"em{s}")
            t["h1"] = wp.tile([P, 64], F32, name=f"h1_{s}")
            t["h1c"] = wp.tile([P, 64], F32, name=f"h1c{s}")
            t["hT"] = wp.tile([64, P], F32, name=f"hT{s}")
            t["h2b"] = wp.tile([P, W2O], F32, name=f"h2b{s}")
            t["ob"] = wp.tile([P, 40], F32, name=f"ob{s}")
            t["ps1"] = pp.tile([P, W1O], F32, space="PSUM", name=f"ps1_{s}")
            t["psT"] = pp.tile([64, P], F32, space="PSUM", name=f"psT{s}")
            t["ps2"] = pp.tile([P, W2O], F32, space="PSUM", name=f"ps2_{s}")
            S.append(t)

        def a1_body(t, i_col, i_row, rows):
            nc.sync.dma_start(out=t["xc"][:, 0:rows], in_=xT[:, i_col])
            nc.tensor.matmul(out=t["ps1"][0:rows, :], lhsT=t["xc"][:, 0:rows],
                             rhs=w1_sb[:], start=True, stop=True)
            nc.scalar.activation(out=t["hb"][0:rows, :], in_=t["ps1"][0:rows, :],
                                 func=AF.Copy)
            nc.sync.dma_start(out=t1_shard[i_row, 0:W1O], in_=t["hb"][0:rows, :])

        def edge_body(t, i_row, i_idx, i_mask, rows, table, adtab, tshape,
                      usedw, NH, OD, post, load_im=True):
            """One window of the edge phase. i_* are ds() slices."""
            HC = NH * OD
            acol = usedw - 2 * NH
            gi, mk, adg = t["gi"], t["mk"], t["adg"]
            ee, ex, msg = t["ee"], t["ex"], t["msg"]
            U, den, rec, agg = t["U"], t["den"], t["rec"], t["agg"]
            gw = t["g"][:].rearrange("p (k w) -> p k w", w=tshape)[:, 0:K, :]
            if load_im:
                nc.sync.dma_start(out=gi[:], in_=idx_in[:, i_idx])
                nc.sync.dma_start(out=mk[:], in_=mask_in[:, i_mask])
            nc.sync.dma_start(out=adg[0:rows, 0:2 * NH],
                              in_=adtab[i_row, acol:acol + 2 * NH])
            off = 0
            coloff = 0
            for base, n_list in ((0, splits1), (1, splits2)):
                tab_ap = table[0:HALF, :] if base == 0 else table[HALF:N_NODES, :]
                for n_idx in n_list:
                    nc.gpsimd.dma_gather(
                        out_ap=gw[:, off:off + n_idx // P, :],
                        in_ap=tab_ap,
                        idxs_ap=gi[:, coloff:coloff + n_idx // 16],
                        num_idxs=n_idx, num_idxs_reg=n_idx, elem_size=tshape,
                        queue_num=0)
                    off += n_idx // P
                    coloff += n_idx // 16
            # logits: e = alpha_src[slot] + alpha_dst[d] + mask
            nc.vector.tensor_add(
                out=ee[:, :, 0:NH], in0=gw[:, :, acol:acol + NH],
                in1=adg[:, None, NH:2 * NH].to_broadcast([P, K, NH]))
            nc.vector.tensor_add(
                out=ee[:, :, 0:NH], in0=ee[:, :, 0:NH],
                in1=mk[:, :, None].to_broadcast([P, K, NH]))
            nc.vector.scalar_tensor_tensor(
                out=ee[:, :, 0:NH], in0=ee[:, :, 0:NH], scalar=0.2,
                in1=ee[:, :, 0:NH], op0=OP.mult, op1=OP.max)
            nc.scalar.activation(out=ex[:, :, 0:NH], in_=ee[:, :, 0:NH],
                                 func=AF.Exp)
            # msg = h[slot] * ex ; U/den = reduce over k ; agg = U/den
            nc.vector.tensor_tensor(
                out=msg[:, :, 0:HC].rearrange("p k (h o) -> p k h o", o=OD),
                in0=gw[:, :, 0:HC].rearrange("p k (h o) -> p k h o", o=OD),
                in1=ex[:, :, 0:NH, None].to_broadcast([P, K, NH, OD]),
                op=OP.mult)
            nc.vector.reduce_sum(
                out=U[:, 0:HC, None],
                in_=msg[:, :, 0:HC].rearrange("p k f -> p f k"),
                axis=mybir.AxisListType.X)
            nc.vector.reduce_sum(
                out=den[:, 0:NH, None],
                in_=ex[:, :, 0:NH].rearrange("p k h -> p h k"),
                axis=mybir.AxisListType.X)
            nc.vector.reciprocal(rec[:, 0:NH], den[:, 0:NH])
            nc.vector.tensor_tensor(
                out=agg[:, 0:HC].rearrange("p (h o) -> p h o", o=OD),
                in0=U[:, 0:HC].rearrange("p (h o) -> p h o", o=OD),
                in1=rec[:, 0:NH, None].to_broadcast([P, NH, OD]), op=OP.mult)
            post(t, rows)

        def post1(i_h1):
            def post(t, rows):
                agg, em, h1 = t["agg"], t["em"], t["h1"]
                nc.vector.tensor_add(out=agg[:, 0:64], in0=agg[:, 0:64],
                                     in1=b1_sb[:])
                nc.scalar.activation(out=em[:], in_=agg[:, 0:64], func=AF.Exp)
                nc.vector.tensor_scalar(out=em[:], in0=em[:], scalar1=-1.0,
                                        scalar2=0.0, op0=OP.add, op1=OP.min)
                nc.vector.scalar_tensor_tensor(
                    out=h1[:], in0=agg[:, 0:64], scalar=0.0, in1=em[:],
                    op0=OP.max, op1=OP.add)
                # fused layer-2 row computation: t2_shard = h1 @ W_ext2
                nc.tensor.transpose(out=t["psT"][:], in_=h1[:],
                                    identity=ident_sb[:])
                nc.scalar.activation(out=t["hT"][:], in_=t["psT"][:],
                                     func=AF.Copy)
                nc.tensor.matmul(out=t["ps2"][:], lhsT=t["hT"][:], rhs=w2_sb[:],
                                 start=True, stop=True)
                nc.scalar.activation(out=t["h2b"][:], in_=t["ps2"][:],
                                     func=AF.Copy)
                nc.sync.dma_start(out=t2_shard[i_h1, 0:W2O],
                                  in_=t["h2b"][0:rows, :])
            return post

        def post2(i_out):
            def post(t, rows):
                agg, em, ob = t["agg"], t["em"], t["ob"]
                den, rec = t["den"], t["rec"]
                nc.vector.tensor_add(out=agg[:, 0:40], in0=agg[:, 0:40],
                                     in1=b2_sb[:])
                nc.vector.reduce_max(out=den[:, 0:1, None],
                                     in_=agg[:, None, 0:40],
                                     axis=mybir.AxisListType.X)
                nc.vector.tensor_sub(out=em[:, 0:40], in0=agg[:, 0:40],
                                     in1=den[:, 0:1].to_broadcast([P, 40]))
                nc.scalar.activation(out=ob[:], in_=em[:, 0:40], func=AF.Exp)
                nc.vector.reduce_sum(out=den[:, 1:2, None], in_=ob[:, None, :],
                                     axis=mybir.AxisListType.X)
                nc.scalar.activation(out=rec[:, 0:1], in_=den[:, 1:2],
                                     func=AF.Ln)
                nc.vector.tensor_sub(out=ob[:], in0=em[:, 0:40],
                                     in1=rec[:, 0:1].to_broadcast([P, 40]))
                nc.sync.dma_start(out=out[i_out, :], in_=ob[0:rows, :])
            return post

        TB = (NW - 1) * P  # tail base row (windows 0..47 paired in loops)
        for rep in range(reps):
            # ---- A1 ----
            with tc.For_i(0, TB, 2 * P) as i:
                a1_body(S[0], ds(i, P), ds(i, P), P)
                a1_body(S[1], ds(i + P, P), ds(i + P, P), P)
            a1_body(S[0], ds(TB, TAILR), ds(TB, TAILR), TAILR)
            nc.gpsimd.collective_compute(
                "AllGather", OP.bypass, replica_groups=[list(range(NC))],
                ins=[t1_shard[:]], outs=[t1[:]])

            # ---- B1 (layer-2 row compute fused into post1) ----
            with tc.For_i(0, NW - 1, 2) as i:
                edge_body(S[0], ds(i * P, P), ds(i * ICOL, ICOL), ds(i * K, K),
                          P, t1, t1_shard, T1W, W1O, 8, 8, post1(ds(i * P, P)))
                edge_body(S[1], ds(i * P + P, P), ds(i * ICOL + ICOL, ICOL),
                          ds(i * K + K, K), P, t1, t1_shard, T1W, W1O, 8, 8,
                          post1(ds(i * P + P, P)))
            edge_body(S[0], ds(TB, TAILR), ds((NW - 1) * ICOL, ICOL),
                      ds((NW - 1) * K, K), TAILR, t1, t1_shard, T1W, W1O,
                      8, 8, post1(ds(TB, TAILR)))
            nc.gpsimd.collective_compute(
                "AllGather", OP.bypass, replica_groups=[list(range(NC))],
                ins=[t2_shard[:]], outs=[t2[:]])

            # ---- B2 ----
            with tc.For_i(0, NW - 1, 2) as i:
                edge_body(S[0], ds(i * P, P), ds(i * ICOL, ICOL), ds(i * K, K),
                          P, t2, t2_shard, T2W, W2O, 1, 40, post2(ds(i * P, P)))
                edge_body(S[1], ds(i * P + P, P), ds(i * ICOL + ICOL, ICOL),
                          ds(i * K + K, K), P, t2, t2_shard, T2W, W2O, 1, 40,
                          post2(ds(i * P + P, P)))
            edge_body(S[0], ds(TB, TAILR), ds((NW - 1) * ICOL, ICOL),
                      ds((NW - 1) * K, K), TAILR, t2, t2_shard, T2W, W2O,
                      1, 40, post2(ds(TB, TAILR)))

        pp.release()
        wp.release()
        cp.release()

    nc.compile()
    return nc


_CACHE = {}


def _get_nc(T, reps=1):
    key = (T, reps)
    if key not in _CACHE:
        K1, K2 = T
        _CACHE[key] = build_kernel(K1, K2, reps=reps)
    return _CACHE[key]


def make_in_maps(x, edge_index, W1, a1_src, a1_dst, b1, W2, a2_src, a2_dst, b2,
                 T, N=None):
    K1, K2 = T
    W_ext1, W_ext2 = _fold_params(W1, a1_src, a1_dst, W2, a2_src, a2_dst)
    src = np.asarray(edge_index[0]).astype(np.int64)
    dst = np.asarray(edge_index[1]).astype(np.int64)
    per_core = _prep_edges(src, dst, K1, K2)
    xTf = np.ascontiguousarray(np.asarray(x, np.float32).T)
    shared = {
        "W_ext1": W_ext1, "W_ext2": W_ext2,
        "b1m": np.tile(np.asarray(b1, np.float32)[None, :], (P, 1)),
        "b2m": np.tile(np.asarray(b2, np.float32)[None, :], (P, 1)),
        "ident_in": np.eye(P, dtype=np.float32),
    }
    return [dict(shared, idx_in=ix, mask_in=mk,
                 xT=np.ascontiguousarray(xTf[:, c * SHARD:(c + 1) * SHARD]))
            for c, (ix, mk) in enumerate(per_core)]


def kernel(x, edge_index, W1, a1_src, a1_dst, b1, W2, a2_src, a2_dst, b2,
           reps=1, nc_override=None):
    x = np.asarray(x, np.float32)
    edge_index = np.asarray(edge_index)
    args = [np.asarray(a, np.float32) for a in
            (W1, a1_src, a1_dst, b1, W2, a2_src, a2_dst, b2)]
    T = required_T(edge_index)
    in_maps = make_in_maps(x, edge_index, *args, T)
    nc = nc_override if nc_override is not None else _get_nc(T, reps)
    res = run_bass_kernel_spmd(nc, in_maps, list(range(NC)))
    return np.concatenate([res.results[c]["out"] for c in range(NC)], axis=0)
